# revision 6
# baseline (speedup 1.0000x reference)
"""Trainium2 Bass kernel for nn_AttentionMM (B=64, T=512, E=512), 8 NeuronCores.

Math (align factored away — O(B*T*E) instead of O(B*T^2*E)):
    u1 = tanh(m1 @ W1 + b1)          u2 = tanh(m2 @ W2 + b2)
    g1 = m1^T @ u1                   g2 = m2^T @ u2
    s1 = m1 @ g2                     s2 = m2 @ g1
    a1 = softmax(s1)                 a2 = softmax(s2)
    v1 = m1^T @ a1                   v2 = m2^T @ a2
    out = concat([v1, v2], -1)

Sharding: pure data parallelism, batch dim 64 -> 8 cores x 8 rows each.

Broadcast-dot design (no PE transposes of m at all):
  - m stays in natural layout [t-part, e-free] only; each element is
    loaded from HBM exactly once (DMA ~48us is the roofline).
  - E-contractions (z = m@W, s = m@g_other) are free-dim multiply-reduce
    against a partition-replicated copy of the contraction vector:
      * DVE path: tensor_tensor_reduce with accum_out, reading the
        replica straight from PSUM.
      * Pool+ACT path (GPSIMD cannot touch PSUM or run TensorScalarPtr):
        ACT first drains the replica to SBUF, Pool does the element-wise
        multiply into scratch, ACT accumulates with accum_out.
    Results land directly in column layout [128 t-part, 4 chunks].
  - W replicas are built by a K=1 PE matmul (ones[1,128]^T @ W_row).
    g replicas fall out of the g-matmul itself: the stationary u-column
    is broadcast along its free dim (u_rep[128,128]^T @ m_chunk), so
    every PSUM row of the [128,512] output is g — no separate g row,
    no broadcast matmul, no extra drain.
  - T-contractions (g, v) are PE f32r matvecs streaming the natural
    layout; stationary columns come straight from ACT tanh/exp outputs
    in column layout — no row<->column conversions anywhere.
  - Softmax runs in column space: DVE free-dim max, GPSIMD
    partition_all_reduce(max), ACT exp with per-partition bias and
    accum_out, partition_all_reduce(add); 1/sum is folded into the ACT
    drain of the v row (softmax scale commutes with the weighted sum).
"""

import numpy as np

B, T, E = 64, 512, 512
N_CORES = 8
B_LOC = B // N_CORES          # 8 batch rows per core
TC = 4                        # 512 = 4 chunks of 128 along t

_NC_CACHE = {}


def _use_pool_path(b, s, op):
    """Which (batch, side, op) dot groups run on Pool+ACT instead of DVE.

    12 of 32 groups -> 48 of 128 chunk-dots, balancing DVE ~53us against
    Pool ~42us and ACT ~54us in the cost model.
    """
    if op == "z":
        return s == 0
    return s == 1 and b % 2 == 0


def build_nc(repeat=1):
    import concourse.bacc as bacc
    import concourse.mybir as mybir
    import concourse.tile as tile
    from concourse import masks
    from concourse import bass_isa

    F32 = mybir.dt.float32
    F32R = mybir.dt.float32r
    AF = mybir.ActivationFunctionType
    AX = mybir.AxisListType
    OP = mybir.AluOpType
    RED = bass_isa.ReduceOp

    nc = bacc.Bacc("TRN2", target_bir_lowering=False, debug=False,
                   num_devices=N_CORES)

    m_d = [nc.dram_tensor("m1", [B_LOC, T, E], F32, kind="ExternalInput"),
           nc.dram_tensor("m2", [B_LOC, T, E], F32, kind="ExternalInput")]
    W_d = [nc.dram_tensor("W1", [B_LOC, E, 1], F32, kind="ExternalInput"),
           nc.dram_tensor("W2", [B_LOC, E, 1], F32, kind="ExternalInput")]
    b_d = [nc.dram_tensor("b1", [B_LOC, T], F32, kind="ExternalInput"),
           nc.dram_tensor("b2", [B_LOC, T], F32, kind="ExternalInput")]
    out_d = nc.dram_tensor("out", [B_LOC, 2 * E], F32, kind="ExternalOutput")

    with tile.TileContext(nc) as tc:
        with (
            tc.tile_pool(name="const", bufs=1) as cpool,
            tc.tile_pool(name="mat", bufs=1) as mpool,
            tc.tile_pool(name="vec", bufs=1) as vpool,
            tc.tile_pool(name="ps_bc", bufs=4, space="PSUM") as ps_bc,
            tc.tile_pool(name="ps_row", bufs=2, space="PSUM") as ps_row,
            tc.tile_pool(name="ps_sm", bufs=1, space="PSUM") as ps_sm,
        ):
            identF = cpool.tile([128, 128], F32)
            masks.make_identity(nc, identF[:])
            onesF = cpool.tile([1, 128], F32)
            nc.gpsimd.memset(onesF[:], 1.0)
            onesR = cpool.tile([1, 128], F32R)
            nc.sync.dma_start(onesR[:], onesF[:].bitcast(F32R))

            def dots(b, s, op, M_s, bc_ps, zcol):
                """zcol[:,t] = sum_e(M_s[:,t,:] * replica) for t in 0..3.

                bc_ps: PSUM [128, E] replica of the contraction vector.
                """
                if _use_pool_path(b, s, op):
                    bsb = vpool.tile([128, E], F32, tag=f"bsb{op}{s}",
                                     name=f"bsb{op}{s}", bufs=2)
                    nc.scalar.activation(bsb[:], bc_ps[:], AF.Copy)
                    for t in range(TC):
                        scr = vpool.tile([128, E], F32, tag="scr",
                                         name="scr", bufs=3)
                        nc.gpsimd.tensor_tensor(
                            scr[:], M_s[:, t, :].bitcast(F32), bsb[:],
                            op=OP.mult)
                        nc.scalar.activation(
                            scr[:], scr[:], AF.Copy,
                            accum_out=zcol[:, t : t + 1])
                else:
                    for t in range(TC):
                        dum = vpool.tile([128, 1], F32, tag="dum",
                                         name="dum", bufs=6)
                        nc.vector.scalar_tensor_tensor(
                            dum[:].broadcast_to([128, E]),
                            M_s[:, t, :].bitcast(F32), 0.0, bc_ps[:],
                            op0=OP.bypass, op1=OP.mult,
                            accum_out=zcol[:, t : t + 1])

            import contextlib
            loop_ctx = (tc.For_i(0, repeat, 1) if repeat > 1
                        else contextlib.nullcontext())
            with loop_ctx:
                for b in range(B_LOC):
                    M = [None, None]
                    Wrow = [None, None]
                    brow = [None, None]
                    wb_ps = [None, None]
                    bcol_ps = [None, None]
                    ucol = [None, None]
                    gb_ps = [None, None]
                    scol = [None, None]

                    # ---------------- loads ----------------
                    for s in range(2):
                        Wrow[s] = vpool.tile([1, E], F32R, tag=f"wr{s}", name=f"Wr{s}", bufs=2)
                        nc.sync.dma_start(
                            Wrow[s][:],
                            W_d[s].ap()[b].rearrange("e one -> one e").bitcast(F32R))
                        brow[s] = vpool.tile([TC, 128], F32, tag=f"br{s}", name=f"Br{s}", bufs=2)
                        nc.sync.dma_start(
                            brow[s][:], b_d[s].ap()[b].rearrange("(c p) -> c p", p=128))
                        M[s] = mpool.tile([128, TC, E], F32R, tag=f"m{s}", name=f"M{s}", bufs=3)
                        for t in range(TC):
                            nc.sync.dma_start(
                                M[s][:, t, :],
                                m_d[s].ap()[b].rearrange(
                                    "(k p) e -> p k e", p=128
                                ).bitcast(F32R)[:, t, :],
                            )

                    # -------- W replica + bias column --------
                    for s in range(2):
                        wb_ps[s] = ps_bc.tile([128, E], F32, tag="bc", name="wb_ps")
                        nc.tensor.matmul(
                            wb_ps[s][:], onesR[:], Wrow[s][:],
                            start=True, stop=True)
                        bcol_ps[s] = ps_sm.tile([128, TC], F32, tag="sm", name="bcol_ps")
                        nc.tensor.transpose(
                            bcol_ps[s][:], brow[s][:], identF[0:TC, 0:TC])

                    # -------- z = m @ W ; u = tanh(z + b) --------
                    for s in range(2):
                        zcol = vpool.tile([128, TC], F32, tag=f"z{s}", name=f"Z{s}", bufs=2)
                        dots(b, s, "z", M[s], wb_ps[s], zcol)
                        zb = vpool.tile([128, TC], F32, tag=f"zb{s}", name=f"Zb{s}", bufs=2)
                        nc.vector.tensor_add(zb[:], zcol[:], bcol_ps[s][:])
                        ucol[s] = vpool.tile([128, TC], F32R, tag=f"u{s}", name=f"U{s}", bufs=2)
                        nc.scalar.activation(ucol[s][:], zb[:], AF.Tanh)

                    # -------- g replica: u_rep^T @ m accumulated in PSUM --------
                    for s in range(2):
                        gb_ps[s] = ps_bc.tile([128, E], F32, tag="bc", name="gb_ps")
                        for t in range(TC):
                            nc.tensor.matmul(
                                gb_ps[s][:],
                                ucol[s][:, t : t + 1].broadcast_to([128, 128]),
                                M[s][:, t, :],
                                start=(t == 0), stop=(t == TC - 1))

                    # -------- s = m @ g_other --------
                    for s in range(2):
                        scol[s] = vpool.tile([128, TC], F32, tag=f"s{s}", name=f"S{s}", bufs=2)
                        dots(b, s, "s", M[s], gb_ps[1 - s], scol[s])

                    # -------- softmax (column space) + v + out --------
                    for s in range(2):
                        mx = vpool.tile([128, 1], F32, tag=f"mx{s}", name=f"Mx{s}", bufs=2)
                        nc.vector.tensor_reduce(
                            mx[:], scol[s][:], axis=AX.X, op=OP.max)
                        mxg = vpool.tile([128, 1], F32, tag=f"mxg{s}", name=f"Mxg{s}", bufs=2)
                        nc.gpsimd.partition_all_reduce(mxg[:], mx[:], 128, RED.max)
                        negm = vpool.tile([128, 1], F32, tag=f"ng{s}", name=f"Ng{s}", bufs=2)
                        nc.scalar.activation(negm[:], mxg[:], AF.Copy, scale=-1.0)
                        pcol = vpool.tile([128, TC], F32R, tag=f"p{s}", name=f"P{s}", bufs=2)
                        sump = vpool.tile([128, 1], F32, tag=f"sp{s}", name=f"Sp{s}", bufs=2)
                        nc.scalar.activation(
                            pcol[:], scol[s][:], AF.Exp, bias=negm[:],
                            accum_out=sump[:])
                        sumg = vpool.tile([128, 1], F32, tag=f"sg{s}", name=f"Sg{s}", bufs=2)
                        nc.gpsimd.partition_all_reduce(sumg[:], sump[:], 128, RED.add)
                        rs = vpool.tile([1, 1], F32, tag=f"rs{s}", name=f"Rs{s}", bufs=2)
                        nc.vector.reciprocal(rs[:], sumg[0:1, :])

                        v_ps = ps_row.tile([1, E], F32, tag="row", name="v_ps")
                        for t in range(TC):
                            nc.tensor.matmul(
                                v_ps[:], pcol[:, t : t + 1],
                                M[s][:, t, :],
                                start=(t == 0), stop=(t == TC - 1))
                        vout = vpool.tile([1, E], F32, tag=f"vo{s}", name=f"Vo{s}", bufs=2)
                        nc.scalar.activation(
                            vout[:], v_ps[:], AF.Copy, scale=rs[:])
                        nc.sync.dma_start(
                            out_d.ap()[b : b + 1, s * E : (s + 1) * E], vout[:])

    nc.compile()
    return nc


def _get_nc(repeat=1):
    if repeat not in _NC_CACHE:
        _NC_CACHE[repeat] = build_nc(repeat)
    return _NC_CACHE[repeat]


def kernel(m1, m2, W1, b1, W2, b2):
    from concourse.bass_utils import run_bass_kernel_spmd

    nc = _get_nc()
    in_maps = []
    for c in range(N_CORES):
        sl = slice(c * B_LOC, (c + 1) * B_LOC)
        in_maps.append({
            "m1": np.ascontiguousarray(m1[sl]),
            "m2": np.ascontiguousarray(m2[sl]),
            "W1": np.ascontiguousarray(W1[sl]),
            "b1": np.ascontiguousarray(b1[sl]),
            "W2": np.ascontiguousarray(W2[sl]),
            "b2": np.ascontiguousarray(b2[sl]),
        })
    res = run_bass_kernel_spmd(nc, in_maps, core_ids=list(range(N_CORES)))
    return np.concatenate([r["out"] for r in res.results], axis=0)


# revision 7
# speedup vs baseline: 2.0977x; 2.0977x over previous
"""Trainium2 Bass kernel for nn_AttentionMM (B=64, T=512, E=512), 8 NeuronCores.

Math (align factored away — O(B*T*E) instead of O(B*T^2*E)):
    u1 = tanh(m1 @ W1 + b1)          u2 = tanh(m2 @ W2 + b2)
    g1 = m1^T @ u1                   g2 = m2^T @ u2
    s1 = m1 @ g2                     s2 = m2 @ g1
    a1 = softmax(s1)                 a2 = softmax(s2)
    v1 = m1^T @ a1                   v2 = m2^T @ a2
    out = concat([v1, v2], -1)

Sharding: pure data parallelism, batch dim 64 -> 8 cores x 8 rows each.

Broadcast-dot design (no PE transposes of m at all):
  - m stays in natural layout [t-part, e-free] only; each element is
    loaded from HBM exactly once (DMA ~48us is the roofline).
  - E-contractions (z = m@W, s = m@g_other) are free-dim multiply-reduce
    against a partition-replicated copy of the contraction vector:
      * DVE path: tensor_tensor_reduce with accum_out, reading the
        replica straight from PSUM.
      * Pool+ACT path (GPSIMD cannot touch PSUM or run TensorScalarPtr):
        ACT first drains the replica to SBUF, Pool does the element-wise
        multiply into scratch, ACT accumulates with accum_out.
    Results land directly in column layout [128 t-part, 4 chunks].
  - W replicas are built by a K=1 PE matmul (ones[1,128]^T @ W_row).
    g replicas fall out of the g-matmul itself: the stationary u-column
    is broadcast along its free dim (u_rep[128,128]^T @ m_chunk), so
    every PSUM row of the [128,512] output is g — no separate g row,
    no broadcast matmul, no extra drain.
  - T-contractions (g, v) are PE f32r matvecs streaming the natural
    layout; stationary columns come straight from ACT tanh/exp outputs
    in column layout — no row<->column conversions anywhere.
  - Softmax runs in column space: DVE free-dim max, GPSIMD
    partition_all_reduce(max), ACT exp with per-partition bias and
    accum_out, partition_all_reduce(add); 1/sum is folded into the ACT
    drain of the v row (softmax scale commutes with the weighted sum).
"""

import numpy as np

B, T, E = 64, 512, 512
N_CORES = 8
B_LOC = B // N_CORES          # 8 batch rows per core
TC = 4                        # 512 = 4 chunks of 128 along t

_NC_CACHE = {}


def _use_pool_path(b, s, op):
    """Which (batch, side, op) dot groups run on Pool+ACT instead of DVE.

    12 of 32 groups -> 48 of 128 chunk-dots, balancing DVE ~53us against
    Pool ~42us and ACT ~54us in the cost model.
    """
    import os
    mode = os.environ.get("DOT_MODE", "mixed")
    if mode == "alldve":
        return False
    if op == "z":
        return s == 0
    return s == 1 and b % 2 == 0


def build_nc(repeat=1):
    import concourse.bacc as bacc
    import concourse.mybir as mybir
    import concourse.tile as tile
    from concourse import masks
    from concourse import bass_isa

    F32 = mybir.dt.float32
    F32R = mybir.dt.float32r
    AF = mybir.ActivationFunctionType
    AX = mybir.AxisListType
    OP = mybir.AluOpType
    RED = bass_isa.ReduceOp

    nc = bacc.Bacc("TRN2", target_bir_lowering=False, debug=False,
                   num_devices=N_CORES)

    m_d = [nc.dram_tensor("m1", [B_LOC, T, E], F32, kind="ExternalInput"),
           nc.dram_tensor("m2", [B_LOC, T, E], F32, kind="ExternalInput")]
    W_d = [nc.dram_tensor("W1", [B_LOC, E, 1], F32, kind="ExternalInput"),
           nc.dram_tensor("W2", [B_LOC, E, 1], F32, kind="ExternalInput")]
    b_d = [nc.dram_tensor("b1", [B_LOC, T], F32, kind="ExternalInput"),
           nc.dram_tensor("b2", [B_LOC, T], F32, kind="ExternalInput")]
    out_d = nc.dram_tensor("out", [B_LOC, 2 * E], F32, kind="ExternalOutput")

    with tile.TileContext(nc) as tc:
        with (
            tc.tile_pool(name="const", bufs=1) as cpool,
            tc.tile_pool(name="mat", bufs=1) as mpool,
            tc.tile_pool(name="vec", bufs=1) as vpool,
            tc.tile_pool(name="ps_bc", bufs=4, space="PSUM") as ps_bc,
            tc.tile_pool(name="ps_row", bufs=2, space="PSUM") as ps_row,
            tc.tile_pool(name="ps_sm", bufs=1, space="PSUM") as ps_sm,
        ):
            identF = cpool.tile([128, 128], F32)
            masks.make_identity(nc, identF[:])
            onesF = cpool.tile([1, 128], F32)
            nc.gpsimd.memset(onesF[:], 1.0)
            onesR = cpool.tile([1, 128], F32R)
            nc.sync.dma_start(onesR[:], onesF[:].bitcast(F32R))

            def dots(b, s, op, M_s, bc_ps, zcol):
                """zcol[:,t] = sum_e(M_s[:,t,:] * replica) for t in 0..3.

                bc_ps: PSUM [128, E] replica of the contraction vector.
                """
                if _use_pool_path(b, s, op):
                    bsb = vpool.tile([128, E], F32, tag=f"bsb{op}{s}",
                                     name=f"bsb{op}{s}", bufs=2)
                    nc.scalar.activation(bsb[:], bc_ps[:], AF.Copy)
                    for t in range(TC):
                        scr = vpool.tile([128, E], F32, tag="scr",
                                         name="scr", bufs=3)
                        nc.gpsimd.tensor_tensor(
                            scr[:], M_s[:, t, :].bitcast(F32), bsb[:],
                            op=OP.mult)
                        nc.scalar.activation(
                            scr[:], scr[:], AF.Copy,
                            accum_out=zcol[:, t : t + 1])
                else:
                    for t in range(TC):
                        dum = vpool.tile([128, 1], F32, tag="dum",
                                         name="dum", bufs=6)
                        nc.vector.scalar_tensor_tensor(
                            dum[:].broadcast_to([128, E]),
                            M_s[:, t, :].bitcast(F32), 0.0, bc_ps[:],
                            op0=OP.bypass, op1=OP.mult,
                            accum_out=zcol[:, t : t + 1])

            import contextlib
            loop_ctx = (tc.For_i(0, repeat, 1) if repeat > 1
                        else contextlib.nullcontext())
            with loop_ctx:
                for b in range(B_LOC):
                    M = [None, None]
                    Wrow = [None, None]
                    brow = [None, None]
                    wb_ps = [None, None]
                    bcol_ps = [None, None]
                    ucol = [None, None]
                    gb_ps = [None, None]
                    scol = [None, None]

                    # ---------------- loads ----------------
                    for s in range(2):
                        Wrow[s] = vpool.tile([1, E], F32R, tag=f"wr{s}", name=f"Wr{s}", bufs=2)
                        nc.sync.dma_start(
                            Wrow[s][:],
                            W_d[s].ap()[b].rearrange("e one -> one e").bitcast(F32R))
                        brow[s] = vpool.tile([TC, 128], F32, tag=f"br{s}", name=f"Br{s}", bufs=2)
                        nc.sync.dma_start(
                            brow[s][:], b_d[s].ap()[b].rearrange("(c p) -> c p", p=128))
                        M[s] = mpool.tile([128, TC, E], F32R, tag=f"m{s}", name=f"M{s}", bufs=3)
                        for t in range(TC):
                            nc.sync.dma_start(
                                M[s][:, t, :],
                                m_d[s].ap()[b].rearrange(
                                    "(k p) e -> p k e", p=128
                                ).bitcast(F32R)[:, t, :],
                            )

                    # -------- W replica + bias column --------
                    for s in range(2):
                        wb_ps[s] = ps_bc.tile([128, E], F32, tag="bc", name="wb_ps")
                        nc.tensor.matmul(
                            wb_ps[s][:], onesR[:], Wrow[s][:],
                            start=True, stop=True)
                        bcol_ps[s] = ps_sm.tile([128, TC], F32, tag="sm", name="bcol_ps")
                        nc.tensor.transpose(
                            bcol_ps[s][:], brow[s][:], identF[0:TC, 0:TC])

                    # -------- z = m @ W ; u = tanh(z + b) --------
                    for s in range(2):
                        zcol = vpool.tile([128, TC], F32, tag=f"z{s}", name=f"Z{s}", bufs=2)
                        dots(b, s, "z", M[s], wb_ps[s], zcol)
                        zb = vpool.tile([128, TC], F32, tag=f"zb{s}", name=f"Zb{s}", bufs=2)
                        nc.vector.tensor_add(zb[:], zcol[:], bcol_ps[s][:])
                        ucol[s] = vpool.tile([128, TC], F32R, tag=f"u{s}", name=f"U{s}", bufs=2)
                        nc.scalar.activation(ucol[s][:], zb[:], AF.Tanh)

                    # -------- g replica: u_rep^T @ m accumulated in PSUM --------
                    for s in range(2):
                        gb_ps[s] = ps_bc.tile([128, E], F32, tag="bc", name="gb_ps")
                        for t in range(TC):
                            nc.tensor.matmul(
                                gb_ps[s][:],
                                ucol[s][:, t : t + 1].broadcast_to([128, 128]),
                                M[s][:, t, :],
                                start=(t == 0), stop=(t == TC - 1))

                    # -------- s = m @ g_other --------
                    for s in range(2):
                        scol[s] = vpool.tile([128, TC], F32, tag=f"s{s}", name=f"S{s}", bufs=2)
                        dots(b, s, "s", M[s], gb_ps[1 - s], scol[s])

                    # -------- softmax (column space) + v + out --------
                    for s in range(2):
                        mx = vpool.tile([128, 1], F32, tag=f"mx{s}", name=f"Mx{s}", bufs=2)
                        nc.vector.tensor_reduce(
                            mx[:], scol[s][:], axis=AX.X, op=OP.max)
                        mxg = vpool.tile([128, 1], F32, tag=f"mxg{s}", name=f"Mxg{s}", bufs=2)
                        nc.gpsimd.partition_all_reduce(mxg[:], mx[:], 128, RED.max)
                        negm = vpool.tile([128, 1], F32, tag=f"ng{s}", name=f"Ng{s}", bufs=2)
                        nc.scalar.activation(negm[:], mxg[:], AF.Copy, scale=-1.0)
                        pcol = vpool.tile([128, TC], F32R, tag=f"p{s}", name=f"P{s}", bufs=2)
                        sump = vpool.tile([128, 1], F32, tag=f"sp{s}", name=f"Sp{s}", bufs=2)
                        nc.scalar.activation(
                            pcol[:], scol[s][:], AF.Exp, bias=negm[:],
                            accum_out=sump[:])
                        sumg = vpool.tile([128, 1], F32, tag=f"sg{s}", name=f"Sg{s}", bufs=2)
                        nc.gpsimd.partition_all_reduce(sumg[:], sump[:], 128, RED.add)
                        rs = vpool.tile([1, 1], F32, tag=f"rs{s}", name=f"Rs{s}", bufs=2)
                        nc.vector.reciprocal(rs[:], sumg[0:1, :])

                        v_ps = ps_row.tile([1, E], F32, tag="row", name="v_ps")
                        for t in range(TC):
                            nc.tensor.matmul(
                                v_ps[:], pcol[:, t : t + 1],
                                M[s][:, t, :],
                                start=(t == 0), stop=(t == TC - 1))
                        vout = vpool.tile([1, E], F32, tag=f"vo{s}", name=f"Vo{s}", bufs=2)
                        nc.scalar.activation(
                            vout[:], v_ps[:], AF.Copy, scale=rs[:])
                        nc.sync.dma_start(
                            out_d.ap()[b : b + 1, s * E : (s + 1) * E], vout[:])

    nc.compile()
    return nc


def _get_nc(repeat=1):
    if repeat not in _NC_CACHE:
        _NC_CACHE[repeat] = build_nc(repeat)
    return _NC_CACHE[repeat]


def kernel(m1, m2, W1, b1, W2, b2):
    from concourse.bass_utils import run_bass_kernel_spmd

    nc = _get_nc()
    in_maps = []
    for c in range(N_CORES):
        sl = slice(c * B_LOC, (c + 1) * B_LOC)
        in_maps.append({
            "m1": np.ascontiguousarray(m1[sl]),
            "m2": np.ascontiguousarray(m2[sl]),
            "W1": np.ascontiguousarray(W1[sl]),
            "b1": np.ascontiguousarray(b1[sl]),
            "W2": np.ascontiguousarray(W2[sl]),
            "b2": np.ascontiguousarray(b2[sl]),
        })
    res = run_bass_kernel_spmd(nc, in_maps, core_ids=list(range(N_CORES)))
    return np.concatenate([r["out"] for r in res.results], axis=0)


# revision 10
# speedup vs baseline: 2.2437x; 1.0696x over previous
"""Trainium2 Bass kernel for nn_AttentionMM (B=64, T=512, E=512), 8 NeuronCores.

Math (align factored away — O(B*T*E) instead of O(B*T^2*E)):
    u1 = tanh(m1 @ W1 + b1)          u2 = tanh(m2 @ W2 + b2)
    g1 = m1^T @ u1                   g2 = m2^T @ u2
    s1 = m1 @ g2                     s2 = m2 @ g1
    a1 = softmax(s1)                 a2 = softmax(s2)
    v1 = m1^T @ a1                   v2 = m2^T @ a2
    out = concat([v1, v2], -1)

Sharding: pure data parallelism, batch dim 64 -> 8 cores x 8 rows each.

Broadcast-dot design (no PE transposes of m at all):
  - m stays in natural layout [t-part, e-free] only; each element is
    loaded from HBM exactly once (DMA ~48us is the roofline).
  - E-contractions (z = m@W, s = m@g_other) are free-dim multiply-reduce
    against a partition-replicated copy of the contraction vector:
      * DVE path: tensor_tensor_reduce with accum_out, reading the
        replica straight from PSUM.
      * Pool+ACT path (GPSIMD cannot touch PSUM or run TensorScalarPtr):
        ACT first drains the replica to SBUF, Pool does the element-wise
        multiply into scratch, ACT accumulates with accum_out.
    Results land directly in column layout [128 t-part, 4 chunks].
  - W replicas are built by a K=1 PE matmul (ones[1,128]^T @ W_row).
    g replicas fall out of the g-matmul itself: the stationary u-column
    is broadcast along its free dim (u_rep[128,128]^T @ m_chunk), so
    every PSUM row of the [128,512] output is g — no separate g row,
    no broadcast matmul, no extra drain.
  - T-contractions (g, v) are PE f32r matvecs streaming the natural
    layout; stationary columns come straight from ACT tanh/exp outputs
    in column layout — no row<->column conversions anywhere.
  - Softmax runs in column space: DVE free-dim max, GPSIMD
    partition_all_reduce(max), ACT exp with per-partition bias and
    accum_out, partition_all_reduce(add); 1/sum is folded into the ACT
    drain of the v row (softmax scale commutes with the weighted sum).
"""

import numpy as np

B, T, E = 64, 512, 512
N_CORES = 8
B_LOC = B // N_CORES          # 8 batch rows per core
TC = 4                        # 512 = 4 chunks of 128 along t

_NC_CACHE = {}


def _use_pool_path(b, s, op):
    """Which (batch, side, op) dot groups run on Pool+ACT instead of DVE.

    12 of 32 groups -> 48 of 128 chunk-dots, balancing DVE ~53us against
    Pool ~42us and ACT ~54us in the cost model.
    """
    import os
    mode = os.environ.get("DOT_MODE", "mixed")
    if mode == "alldve":
        return False
    if op == "z":
        return s == 0
    return s == 1 and b % 2 == 0


def build_nc(repeat=1):
    import concourse.bacc as bacc
    import concourse.mybir as mybir
    import concourse.tile as tile
    from concourse import masks
    from concourse import bass_isa

    F32 = mybir.dt.float32
    F32R = mybir.dt.float32r
    AF = mybir.ActivationFunctionType
    AX = mybir.AxisListType
    OP = mybir.AluOpType
    RED = bass_isa.ReduceOp

    nc = bacc.Bacc("TRN2", target_bir_lowering=False, debug=False,
                   num_devices=N_CORES)

    m_d = [nc.dram_tensor("m1", [B_LOC, T, E], F32, kind="ExternalInput"),
           nc.dram_tensor("m2", [B_LOC, T, E], F32, kind="ExternalInput")]
    W_d = [nc.dram_tensor("W1", [B_LOC, E, 1], F32, kind="ExternalInput"),
           nc.dram_tensor("W2", [B_LOC, E, 1], F32, kind="ExternalInput")]
    b_d = [nc.dram_tensor("b1", [B_LOC, T], F32, kind="ExternalInput"),
           nc.dram_tensor("b2", [B_LOC, T], F32, kind="ExternalInput")]
    out_d = nc.dram_tensor("out", [B_LOC, 2 * E], F32, kind="ExternalOutput")

    with tile.TileContext(nc) as tc:
        with (
            tc.tile_pool(name="const", bufs=1) as cpool,
            tc.tile_pool(name="mat", bufs=1) as mpool,
            tc.tile_pool(name="vec", bufs=1) as vpool,
            tc.tile_pool(name="ps_bc", bufs=3, space="PSUM") as ps_bc,
            tc.tile_pool(name="ps_row", bufs=2, space="PSUM") as ps_row,
            tc.tile_pool(name="ps_sm", bufs=1, space="PSUM") as ps_sm,
        ):
            identF = cpool.tile([128, 128], F32)
            masks.make_identity(nc, identF[:])
            onesF = cpool.tile([1, 128], F32)
            nc.gpsimd.memset(onesF[:], 1.0)
            onesR = cpool.tile([1, 128], F32R)
            nc.sync.dma_start(onesR[:], onesF[:].bitcast(F32R))

            def dots(b, s, op, M_s, bc_ps, zcol):
                """zcol[:,t] = sum_e(M_s[:,t,:] * replica) for t in 0..3.

                bc_ps: PSUM [128, E] replica of the contraction vector.
                """
                if _use_pool_path(b, s, op):
                    bsb = vpool.tile([128, E], F32, tag=f"bsb{op}{s}",
                                     name=f"bsb{op}{s}", bufs=2)
                    nc.scalar.activation(bsb[:], bc_ps[:], AF.Copy)
                    for t in range(TC):
                        scr = vpool.tile([128, E], F32, tag="scr",
                                         name="scr", bufs=3)
                        nc.gpsimd.tensor_tensor(
                            scr[:], M_s[:, t, :].bitcast(F32), bsb[:],
                            op=OP.mult)
                        nc.scalar.activation(
                            scr[:], scr[:], AF.Copy,
                            accum_out=zcol[:, t : t + 1])
                else:
                    for t in range(TC):
                        dum = vpool.tile([128, 1], F32, tag="dum",
                                         name="dum", bufs=6)
                        nc.vector.scalar_tensor_tensor(
                            dum[:].broadcast_to([128, E]),
                            M_s[:, t, :].bitcast(F32), 0.0, bc_ps[:],
                            op0=OP.bypass, op1=OP.mult,
                            accum_out=zcol[:, t : t + 1])

            import contextlib
            loop_ctx = (tc.For_i(0, repeat, 1) if repeat > 1
                        else contextlib.nullcontext())
            with loop_ctx:
                for b in range(B_LOC):
                    M = [None, None]
                    Wrow = [None, None]
                    brow = [None, None]
                    wb_ps = [None, None]
                    bcol_ps = [None, None]
                    ucol = [None, None]
                    gb_ps = [None, None]
                    scol = [None, None]

                    # ---------------- loads ----------------
                    for s in range(2):
                        Wrow[s] = vpool.tile([1, E], F32R, tag=f"wr{s}", name=f"Wr{s}", bufs=2)
                        nc.sync.dma_start(
                            Wrow[s][:],
                            W_d[s].ap()[b].rearrange("e one -> one e").bitcast(F32R))
                        brow[s] = vpool.tile([TC, 128], F32, tag=f"br{s}", name=f"Br{s}", bufs=2)
                        nc.sync.dma_start(
                            brow[s][:], b_d[s].ap()[b].rearrange("(c p) -> c p", p=128))
                        M[s] = mpool.tile([128, TC, E], F32R, tag=f"m{s}", name=f"M{s}", bufs=3)
                        for t in range(TC):
                            nc.sync.dma_start(
                                M[s][:, t, :],
                                m_d[s].ap()[b].rearrange(
                                    "(k p) e -> p k e", p=128
                                ).bitcast(F32R)[:, t, :],
                            )

                    # -------- W replica + bias column --------
                    for s in range(2):
                        wb_ps[s] = ps_bc.tile([128, E], F32, tag="bc", name="wb_ps")
                        nc.tensor.matmul(
                            wb_ps[s][:], onesR[:], Wrow[s][:],
                            start=True, stop=True)
                        bcol_ps[s] = ps_sm.tile([128, TC], F32, tag="sm", name="bcol_ps")
                        nc.tensor.transpose(
                            bcol_ps[s][:], brow[s][:], identF[0:TC, 0:TC])

                    # -------- z = m @ W ; u = tanh(z + b) --------
                    for s in range(2):
                        zcol = vpool.tile([128, TC], F32, tag=f"z{s}", name=f"Z{s}", bufs=2)
                        dots(b, s, "z", M[s], wb_ps[s], zcol)
                        zb = vpool.tile([128, TC], F32, tag=f"zb{s}", name=f"Zb{s}", bufs=2)
                        nc.vector.tensor_add(zb[:], zcol[:], bcol_ps[s][:])
                        ucol[s] = vpool.tile([128, TC], F32R, tag=f"u{s}", name=f"U{s}", bufs=2)
                        nc.scalar.activation(ucol[s][:], zb[:], AF.Tanh)

                    # -------- g replica: u_rep^T @ m accumulated in PSUM --------
                    for s in range(2):
                        gb_ps[s] = ps_bc.tile([128, E], F32, tag="bc", name="gb_ps")
                        for t in range(TC):
                            nc.tensor.matmul(
                                gb_ps[s][:],
                                ucol[s][:, t : t + 1].broadcast_to([128, 128]),
                                M[s][:, t, :],
                                start=(t == 0), stop=(t == TC - 1))

                    # -------- s = m @ g_other --------
                    for s in range(2):
                        scol[s] = vpool.tile([128, TC], F32, tag=f"s{s}", name=f"S{s}", bufs=2)
                        dots(b, s, "s", M[s], gb_ps[1 - s], scol[s])

                    # -------- softmax (column space, PE-assisted reductions) --------
                    for s in range(2):
                        mx = vpool.tile([128, 1], F32, tag=f"mx{s}", name=f"Mx{s}", bufs=2)
                        nc.vector.tensor_reduce(
                            mx[:], scol[s][:], axis=AX.X, op=OP.max)
                        mxT_ps = ps_sm.tile([1, 128], F32, tag="smT", name="mxT_ps")
                        nc.tensor.transpose(mxT_ps[:], mx[:], identF[:])
                        ngs = vpool.tile([1, 1], F32, tag=f"ngs{s}", name=f"Ngs{s}", bufs=2)
                        nc.vector.tensor_reduce(
                            ngs[:], mxT_ps[:], axis=AX.X, op=OP.max, negate=True)
                        negm_ps = ps_sm.tile([128, 1], F32, tag="smB", name="negm_ps")
                        nc.tensor.matmul(
                            negm_ps[:], onesF[:], ngs[:], start=True, stop=True)
                        negm = vpool.tile([128, 1], F32, tag=f"ng{s}", name=f"Ng{s}", bufs=2)
                        nc.vector.tensor_copy(negm[:], negm_ps[:])
                        pcol = vpool.tile([128, TC], F32R, tag=f"p{s}", name=f"P{s}", bufs=2)
                        sump = vpool.tile([128, 1], F32, tag=f"sp{s}", name=f"Sp{s}", bufs=2)
                        nc.scalar.activation(
                            pcol[:], scol[s][:], AF.Exp, bias=negm[:],
                            accum_out=sump[:])
                        spT_ps = ps_sm.tile([1, 128], F32, tag="smT", name="spT_ps")
                        nc.tensor.transpose(spT_ps[:], sump[:], identF[:])
                        ssum = vpool.tile([1, 1], F32, tag=f"ss{s}", name=f"Ss{s}", bufs=2)
                        nc.vector.tensor_reduce(
                            ssum[:], spT_ps[:], axis=AX.X, op=OP.add)
                        rs = vpool.tile([1, 1], F32, tag=f"rs{s}", name=f"Rs{s}", bufs=2)
                        nc.vector.reciprocal(rs[:], ssum[:])

                        v_ps = ps_row.tile([1, E], F32, tag="row", name="v_ps")
                        for t in range(TC):
                            nc.tensor.matmul(
                                v_ps[:], pcol[:, t : t + 1],
                                M[s][:, t, :],
                                start=(t == 0), stop=(t == TC - 1))
                        vout = vpool.tile([1, E], F32, tag=f"vo{s}", name=f"Vo{s}", bufs=2)
                        nc.scalar.activation(
                            vout[:], v_ps[:], AF.Copy, scale=rs[:])
                        nc.sync.dma_start(
                            out_d.ap()[b : b + 1, s * E : (s + 1) * E], vout[:])

    nc.compile()
    return nc


def _get_nc(repeat=1):
    if repeat not in _NC_CACHE:
        _NC_CACHE[repeat] = build_nc(repeat)
    return _NC_CACHE[repeat]


def kernel(m1, m2, W1, b1, W2, b2):
    from concourse.bass_utils import run_bass_kernel_spmd

    nc = _get_nc()
    in_maps = []
    for c in range(N_CORES):
        sl = slice(c * B_LOC, (c + 1) * B_LOC)
        in_maps.append({
            "m1": np.ascontiguousarray(m1[sl]),
            "m2": np.ascontiguousarray(m2[sl]),
            "W1": np.ascontiguousarray(W1[sl]),
            "b1": np.ascontiguousarray(b1[sl]),
            "W2": np.ascontiguousarray(W2[sl]),
            "b2": np.ascontiguousarray(b2[sl]),
        })
    res = run_bass_kernel_spmd(nc, in_maps, core_ids=list(range(N_CORES)))
    return np.concatenate([r["out"] for r in res.results], axis=0)
